# revision 23
# baseline (speedup 1.0000x reference)
"""CrossAttention Trainium2 kernel (8-core SPMD, batch x head-group sharded).

Problem (hardcoded): x (2,2048,1024) fp32, context (2,2048,1152) fp32,
Wq (1024,1024), Wk/Wv (1024,1152), Wo (1024,1024), zero biases.
16 heads x 64 dim, RoPE (interleaved rotate_half, cat-table), softmax over K,
out projection. Output (2, 2048, 1024) fp32.

Sharding: core c in 0..7 handles batch b = c//4 and head group g = c%4
(heads 4g..4g+3). Each core computes a partial y_c = attn(heads) @ Wo_slice;
host sums 4 partials per batch and adds bo.

v2 design (cost-model driven):
  - bf16 operands everywhere on the attention path (proj inputs/weights,
    q/k/v, softmax weights, attn out, Wo). Matmul cost = out-cols regardless
    of dtype; bf16 halves DMA + SBUF and permits 1024-wide moving operands.
  - RoPE via single projection + R*q rotation matmul on PE (R = 128x128
    block-diag rotate_half matrix) + 3 DVE combine ops per tile.
  - scores s[k128, t1024] = kT_h^T q_h per (head, kt);  exp on ACT (the
    133us critical path);  AV transposed: o[t128, 65] = wt_tile^T v_aug
    (65th v col = ones -> softmax denom Z), accumulated over kt.
  - normalize on DVE (1/Z per t-partition), PE-transpose o -> on[e, t],
    O-proj y[t, m] = on^T Wo, DVE copy, DMA out.
  - emission order software-pipelines everything: projections / V / O-proj
    / transposes are spliced into the exp-paced attention loops as PE
    fillers so PE and ACT both stay busy.
"""
import sys

sys.path.insert(0, "/opt/trn_rl_repo")

import numpy as np
import ml_dtypes
import concourse.bass as bass  # noqa: F401
import concourse.mybir as mybir
import concourse.tile as tile
from concourse import bacc
from concourse.bass_utils import run_bass_kernel_spmd

F32 = mybir.dt.float32
F32R = mybir.dt.float32r
BF16 = mybir.dt.bfloat16
AF = mybir.ActivationFunctionType
OP = mybir.AluOpType
BF = ml_dtypes.bfloat16

DIM = 1024
SRC = 1152
NH = 16
HD = 64
GH = 4          # heads per core
GD = GH * HD    # 256 projection dims per core
ROPE_BASE = 10000.0


# ---------------------------------------------------------------- host helpers
def _rope_tables(seq_len: int, head_dim: int):
    inv_freq = 1.0 / (ROPE_BASE ** (np.arange(0, head_dim, 2, dtype=np.float32) / head_dim))
    t = np.arange(seq_len, dtype=np.float32)
    freqs = t[:, None] * inv_freq[None, :]
    emb = np.concatenate([freqs, freqs], axis=-1)  # (L, 64)
    return np.cos(emb).astype(np.float32), np.sin(emb).astype(np.float32)


def _rot128():
    """128x128 rotate_half matrix for a 2-head partition chunk (interleaved)."""
    r64 = np.zeros((HD, HD), dtype=np.float32)
    for i in range(HD // 2):
        r64[2 * i, 2 * i + 1] = -1.0
        r64[2 * i + 1, 2 * i] = 1.0
    return np.kron(np.eye(2, dtype=np.float32), r64)   # (128, 128)


# ---------------------------------------------------------------- device build
def build_nc(T: int, K: int, n_cores: int = 8):
    assert T % 1024 == 0 and K % 512 == 0
    NTW = T // 1024         # 1024-wide t windows ("tw")
    NKT = K // 128          # 128-wide k tiles
    NKW = K // 512          # 512-wide k/proj windows
    NQW = T // 512          # 512-wide q proj windows
    NCC = DIM // 128        # x contraction chunks (8)
    NCS = SRC // 128        # context contraction chunks (9)

    nc = bacc.Bacc("TRN2", target_bir_lowering=False, debug=False,
                   num_devices=n_cores)

    xT = nc.declare_dram_parameter("xT", [DIM, T], BF16, isOutput=False)
    cT = nc.declare_dram_parameter("cT", [SRC, K], BF16, isOutput=False)
    wqT = nc.declare_dram_parameter("wqT", [DIM, GD], BF16, isOutput=False)
    wkT = nc.declare_dram_parameter("wkT", [SRC, GD], BF16, isOutput=False)
    wvT = nc.declare_dram_parameter("wvT", [SRC, GH * 65], BF16, isOutput=False)
    woT = nc.declare_dram_parameter("woT", [GD, DIM], BF16, isOutput=False)
    rT = nc.declare_dram_parameter("rT", [128, 128], F32R, isOutput=False)
    cosT = nc.declare_dram_parameter("cosT", [128, max(T, K)], BF16, isOutput=False)
    sinT = nc.declare_dram_parameter("sinT", [128, max(T, K)], BF16, isOutput=False)
    idn = nc.declare_dram_parameter("idn", [128, 128], BF16, isOutput=False)
    y = nc.declare_dram_parameter("y", [T, DIM], F32, isOutput=True)

    with tile.TileContext(nc) as tc:
        with (
            tc.tile_pool(name="consts", bufs=1) as consts,
            tc.tile_pool(name="resid", bufs=1) as resid,
            tc.tile_pool(name="rope", bufs=2) as rope,
            tc.tile_pool(name="wtp", bufs=6) as wtp,
            tc.tile_pool(name="osbp", bufs=1) as osbp,
            tc.tile_pool(name="zp", bufs=4) as zp,
            tc.tile_pool(name="yp", bufs=2) as yp,
            tc.tile_pool(name="pp", bufs=2, space="PSUM") as pp,
            tc.tile_pool(name="sp", bufs=2, space="PSUM") as sp,
            tc.tile_pool(name="avp", bufs=1, space="PSUM") as avp,
        ):
            # ---------------- constants (DMA order = arrival order) ---------
            # The serial DMA track gates the pipeline start: order transfers
            # by first use so the first exp fires as early as possible.
            rT_sb = consts.tile([128, 128], F32R, tag="rT")
            nc.sync.dma_start(out=rT_sb, in_=rT[:, :])
            wk_sb = consts.tile([128, NCS, GD], BF16, tag="wk")
            nc.sync.dma_start(out=wk_sb, in_=wkT[:, :].rearrange("(o p) f -> p o f", p=128))
            ct_sb = consts.tile([128, NCS, K], BF16, tag="ct")
            ct_dram = cT[:, :].rearrange("(o p) t -> p o t", p=128)

            def ct_win_dma(w):
                nc.sync.dma_start(out=ct_sb[:, :, w * 512:(w + 1) * 512],
                                  in_=ct_dram[:, :, w * 512:(w + 1) * 512])

            ct_win_dma(0)
            wq_sb = consts.tile([128, NCC, GD], BF16, tag="wq")
            nc.sync.dma_start(out=wq_sb, in_=wqT[:, :].rearrange("(o p) f -> p o f", p=128))
            x_sb = consts.tile([128, NCC, T], BF16, tag="x")
            x_dram = xT[:, :].rearrange("(o p) t -> p o t", p=128)

            def x_win_dma(w):
                nc.sync.dma_start(out=x_sb[:, :, w * 512:(w + 1) * 512],
                                  in_=x_dram[:, :, w * 512:(w + 1) * 512])

            x_win_dma(0)
            x_win_dma(1)
            cos_sb = consts.tile([128, max(T, K)], BF16, tag="cos")
            nc.sync.dma_start(out=cos_sb, in_=cosT[:, :])
            sin_sb = consts.tile([128, max(T, K)], BF16, tag="sin")
            nc.sync.dma_start(out=sin_sb, in_=sinT[:, :])
            wv_sb = consts.tile([128, NCS, GH * 65], BF16, tag="wv")
            nc.sync.dma_start(out=wv_sb, in_=wvT[:, :].rearrange("(o p) f -> p o f", p=128))
            ct_win_dma(1)
            ct_win_dma(2)
            ct_win_dma(3)
            wo_sb = consts.tile([128, 2, DIM], BF16, tag="wo")
            nc.sync.dma_start(out=wo_sb, in_=woT[:, :].rearrange("(o p) f -> p o f", p=128))
            idn_sb = consts.tile([128, 128], BF16, tag="idn")
            nc.sync.dma_start(out=idn_sb, in_=idn[:, :])
            for w in range(2, NQW):
                x_win_dma(w)

            # ---------------- residents ------------------------------------
            kT_sb = [resid.tile([128, K], BF16, tag=f"kT{m}", name=f"kT{m}") for m in range(2)]
            qT_sb = [resid.tile([128, T], BF16, tag=f"qT{m}", name=f"qT{m}") for m in range(2)]
            v_sb = [resid.tile([128, GH * 65], BF16, tag=f"v{kt}", name=f"v{kt}")
                    for kt in range(NKT)]
            on_sb = [resid.tile([128, T], BF16, tag=f"on{m}", name=f"on{m}") for m in range(2)]

            wt = {}     # (h, kt) -> wt tile [128, 1024] for current tw
            osb = {}    # tt -> [128, GD] tile for current tw

            # ---------------- emission helpers -----------------------------
            def rope_combine(dst, raw_sb, n):
                """dst[:, n*512:+512] (bf16) = raw*cos + (R raw)*sin."""
                t0 = n * 512
                rps = pp.tile([128, 512], F32, tag="pps", name="rps")
                nc.tensor.matmul(rps, rT_sb, raw_sb, start=True, stop=True)
                t1 = rope.tile([128, 512], F32, tag="t1")
                nc.vector.tensor_tensor(t1, raw_sb, cos_sb[:, t0:t0 + 512], OP.mult)
                t2 = rope.tile([128, 512], F32, tag="t2")
                nc.vector.tensor_tensor(t2, rps, sin_sb[:, t0:t0 + 512], OP.mult)
                nc.vector.tensor_tensor(dst[:, t0:t0 + 512], t1, t2, OP.add)

            def proj_k(m, n):
                kps = pp.tile([128, 512], F32, tag="pps", name="kps")
                for c in range(NCS):
                    nc.tensor.matmul(kps, wk_sb[:, c, m * 128:(m + 1) * 128],
                                     ct_sb[:, c, n * 512:(n + 1) * 512],
                                     start=(c == 0), stop=(c == NCS - 1))
                ksb = rope.tile([128, 512], F32R, tag="ksb")
                nc.vector.tensor_copy(ksb, kps)
                rope_combine(kT_sb[m], ksb, n)

            def proj_q(m, n):
                qps = pp.tile([128, 512], F32, tag="pps", name="qps")
                for c in range(NCC):
                    nc.tensor.matmul(qps, wq_sb[:, c, m * 128:(m + 1) * 128],
                                     x_sb[:, c, n * 512:(n + 1) * 512],
                                     start=(c == 0), stop=(c == NCC - 1))
                qsb = rope.tile([128, 512], F32R, tag="ksb", name="qsb")
                nc.vector.tensor_copy(qsb, qps)
                rope_combine(qT_sb[m], qsb, n)

            def proj_v(kt):
                vps = pp.tile([128, GH * 65], F32, tag="pps", name="vps")
                for c in range(NCS):
                    nc.tensor.matmul(vps, ct_sb[:, c, kt * 128:(kt + 1) * 128],
                                     wv_sb[:, c, :], start=(c == 0), stop=(c == NCS - 1))
                nc.vector.tensor_copy(v_sb[kt], vps)
                ones_ap = v_sb[kt].rearrange("p (h e) -> p h e", h=GH)[:, :, 64]
                nc.vector.memset(ones_ap, 1.0)

            def emit_scores(tw, h, kt):
                """One scores matmul pair + exp for (head h, t-window tw, k-tile kt)."""
                m, off = h // 2, (h % 2) * 64
                sps = sp.tile([128, 1024], F32, tag="sps", name="sps")
                for half in range(2):
                    nc.tensor.matmul(
                        sps[:, half * 512:(half + 1) * 512],
                        kT_sb[m][off:off + 64, kt * 128:(kt + 1) * 128],
                        qT_sb[m][off:off + 64, tw * 1024 + half * 512:tw * 1024 + (half + 1) * 512],
                        start=True, stop=True)
                wt_t = wtp.tile([128, 1024], BF16, tag="wt", name="wt")
                nc.scalar.activation(wt_t, sps, AF.Exp, scale=0.125)
                wt[(h, kt)] = wt_t

            def emit_av_kt(h, kt, avA, avB):
                """Accumulate o[t, 65] for all 8 tts of one kt (JIT per-kt).

                Tiles are pre-zeroed by DVE memset; matmuls use start=False so
                per-element has_written bits drive accumulate-vs-overwrite.
                """
                for tt in range(8):
                    at = avA if tt < 4 else avB
                    col = (tt % 4) * 65
                    nc.tensor.matmul(
                        at[:, col:col + 65],
                        wt[(h, kt)][:, tt * 128:(tt + 1) * 128],
                        v_sb[kt][:, h * 65:(h + 1) * 65],
                        start=False, stop=(kt == NKT - 1),
                        skip_group_check=True)

            def emit_norm(tw, h, av_tile, tts):
                """osb[tt][:, h-cols] = o / Z for the 4 tts of one AV pass."""
                zsb = zp.tile([128, 4], F32, tag="zsb", name="zsb")
                for j, tt in enumerate(tts):
                    col = (tt % 4) * 65
                    nc.vector.tensor_copy(zsb[:, j:j + 1], av_tile[:, col + 64:col + 65])
                rec = zp.tile([128, 4], F32, tag="rec", name="rec")
                nc.vector.reciprocal(rec, zsb)
                for j, tt in enumerate(tts):
                    if tt not in osb:
                        ot = osbp.tile([128, GD], BF16, tag=f"osb{tt}", name=f"osb{tt}")
                        osb[tt] = ot
                    col = (tt % 4) * 65
                    nc.vector.tensor_scalar(
                        osb[tt][:, h * 64:(h + 1) * 64],
                        av_tile[:, col:col + 64],
                        rec[:, j:j + 1], None, OP.mult)

            def emit_transp(tw, m, tt, ot):
                # transpose psum borrows the short-lived proj/yps bank pair
                tps = pp.tile([128, 128], BF16, tag="pps", name="tps")
                nc.tensor.transpose(tps, ot[:, m * 128:(m + 1) * 128], idn_sb)
                nc.vector.tensor_copy(
                    on_sb[m][:, tw * 1024 + tt * 128:tw * 1024 + (tt + 1) * 128], tps)

            def emit_oproj(tw, tt, use_act=False):
                gtt = tw * 8 + tt
                for nn in range(2):
                    yps = pp.tile([128, 512], F32, tag="pps", name="yps")
                    for dc in range(2):
                        nc.tensor.matmul(yps, on_sb[dc][:, gtt * 128:(gtt + 1) * 128],
                                         wo_sb[:, dc, nn * 512:(nn + 1) * 512],
                                         start=(dc == 0), stop=(dc == 1))
                    yt = yp.tile([128, 512], F32, tag="yt", name="yt")
                    if use_act:
                        nc.scalar.copy(yt, yps)   # ACT is idle post-exp (tail)
                    else:
                        nc.vector.tensor_copy(yt, yps)
                    nc.sync.dma_start(out=y[gtt * 128:(gtt + 1) * 128, nn * 512:(nn + 1) * 512],
                                      in_=yt)

            # ---------------- filler queue ----------------------------------
            # Deferred PE work, consumed in FIFO order with a cycle budget.
            # ensure_*() force-drains through a key so emission order always
            # places producers before consumers (tile deps follow emission
            # order, so this is a correctness requirement, not just pacing).
            fillers = []          # list of (key, cost_cycles, fn)
            done_keys = set()
            debt = [0.0]

            def pop_filler(charge=True):
                key, cost, fn = fillers.pop(0)
                fn()
                done_keys.add(key)
                if charge:
                    debt[0] -= cost

            def emit_fillers(budget):
                # cap so long filler droughts don't bank unbounded credit
                debt[0] = min(debt[0] + budget, 6000.0)
                while fillers and fillers[0][1] <= debt[0]:
                    pop_filler()

            def ensure(keys):
                # forced pops are real PE time already spent out-of-budget;
                # don't let them starve later budget-based draining
                missing = [k for k in keys if k not in done_keys]
                while missing:
                    if not fillers:
                        raise AssertionError(f"unsatisfiable deps: {missing}")
                    pop_filler(charge=False)
                    missing = [k for k in missing if k not in done_keys]

            def q_keys(m, tw):
                return [("Q", m, n) for n in range(2 * tw, 2 * tw + 2)]

            def v_keys():
                return [("V", kt) for kt in range(NKT)]

            def scores_guarded(tw, h, kt):
                m = h // 2
                ensure([("K", m, kt // 4)])
                ensure(q_keys(m, tw))
                emit_scores(tw, h, kt)

            # ---------------- schedule --------------------------------------
            # Prefix: only what the very first scores need.
            proj_k(0, 0)
            done_keys.add(("K", 0, 0))
            for n in range(2):
                proj_q(0, n)
                done_keys.add(("Q", 0, n))

            # Static fillers, interleaved to match DMA arrival order.
            for kt in range(NKT):
                fillers.append((("V", kt), 9 * 260, lambda kt=kt: proj_v(kt)))
                if kt in (2, 4, 6):   # K windows 1..3 land between V batches
                    n = kt // 2
                    fillers.append((("K", 0, n), 10 * 512, lambda n=n: proj_k(0, n)))
            for n in range(NKW):
                fillers.append((("K", 1, n), 10 * 512, lambda n=n: proj_k(1, n)))
            for n in range(2):
                fillers.append((("Q", 1, n), 9 * 512, lambda n=n: proj_q(1, n)))
            for twl in range(1, NTW):
                for m in range(2):
                    for n in range(twl * 2, twl * 2 + 2):
                        fillers.append((("Q", m, n), 9 * 512,
                                        lambda m=m, n=n: proj_q(m, n)))

            heads = [(tw, h) for tw in range(NTW) for h in range(GH)]

            for i, (tw, h) in enumerate(heads):
                avA = avp.tile([128, 4 * 65], F32, tag="avA", name="avA")
                avB = avp.tile([128, 4 * 65], F32, tag="avB", name="avB")
                nc.vector.memset(avA, 0.0)
                nc.vector.memset(avB, 0.0)
                # -- exp-paced loop: scores(kt), AV(kt-1) once its exp is done
                for kt in range(NKT):
                    scores_guarded(tw, h, kt)
                    if kt > 0:
                        ensure([("V", kt - 1)])
                        emit_av_kt(h, kt - 1, avA, avB)
                    emit_fillers(1300)
                ensure([("V", NKT - 1)])
                emit_av_kt(h, NKT - 1, avA, avB)
                emit_norm(tw, h, avA, [0, 1, 2, 3])
                emit_norm(tw, h, avB, [4, 5, 6, 7])
                # -- post-head bookkeeping: transposes + O-proj fillers
                last = tw == NTW - 1
                if h == 1:
                    for tt in range(8):
                        fillers.append((("T", tw, 0, tt), 128,
                                        lambda tw=tw, tt=tt, ot=osb[tt]:
                                        emit_transp(tw, 0, tt, ot)))
                elif h == 3:
                    for tt in range(8):
                        fillers.append((("T", tw, 1, tt), 128,
                                        lambda tw=tw, tt=tt, ot=osb[tt]:
                                        emit_transp(tw, 1, tt, ot)))
                    for tt in range(8):
                        fillers.append((("O", tw, tt), 4 * 512 + 600,
                                        lambda tw=tw, tt=tt, last=last:
                                        emit_oproj(tw, tt, use_act=last)))
                    osb.clear()   # next tw allocates fresh (pool-rotated) tiles

            while fillers:
                pop_filler()

    nc.compile()
    return nc


# ---------------------------------------------------------------- host wrapper
def make_in_maps(x, context, Wq, Wk, Wv, Wo, n_cores=8):
    B, T, _ = x.shape
    K = context.shape[1]
    cos, sin = _rope_tables(max(T, K), HD)      # (L, 64)
    cosT = np.ascontiguousarray(np.tile(cos.T, (2, 1)))   # (128, L)
    sinT = np.ascontiguousarray(np.tile(sin.T, (2, 1)))
    rt = np.ascontiguousarray(_rot128().T)

    in_maps = []
    for c in range(n_cores):
        b, g = c // 4, c % 4
        sl = slice(g * GD, (g + 1) * GD)
        wvTa = np.zeros((SRC, GH * 65), dtype=np.float32)
        for h in range(GH):
            wvTa[:, h * 65:h * 65 + 64] = Wv[g * GD + h * HD: g * GD + (h + 1) * HD, :].T
        in_maps.append({
            "xT": np.ascontiguousarray(x[b].T).astype(BF),
            "cT": np.ascontiguousarray(context[b].T).astype(BF),
            "wqT": np.ascontiguousarray(Wq[sl, :].T).astype(BF),
            "wkT": np.ascontiguousarray(Wk[sl, :].T).astype(BF),
            "wvT": wvTa.astype(BF),
            "woT": np.ascontiguousarray(Wo[:, sl].T).astype(BF),
            "rT": rt,
            "cosT": cosT.astype(BF), "sinT": sinT.astype(BF),
            "idn": np.eye(128, dtype=np.float32).astype(BF),
        })
    return in_maps


def run(nc, in_maps, n_cores=8):
    res = run_bass_kernel_spmd(nc, in_maps, core_ids=list(range(n_cores)))
    return res.results


def kernel(x, context, Wq, bq, Wk, bk, Wv, bv, Wo, bo):
    B, T, _ = x.shape
    K = context.shape[1]
    x = np.asarray(x, dtype=np.float32)
    context = np.asarray(context, dtype=np.float32)
    Wq, Wk, Wv, Wo = (np.asarray(a, dtype=np.float32) for a in (Wq, Wk, Wv, Wo))
    bq, bk, bv, bo = (np.asarray(a, dtype=np.float32) for a in (bq, bk, bv, bo))

    nc = build_nc(T, K, n_cores=8)
    in_maps = make_in_maps(x, context, Wq, Wk, Wv, Wo)
    assert not bq.any() and not bk.any() and not bv.any(), "nonzero qkv bias unsupported"
    results = run(nc, in_maps)

    out = np.zeros((B, T, DIM), dtype=np.float32)
    for c in range(8):
        out[c // 4] += results[c]["y"]
    out += bo[None, None, :]
    return out


if __name__ == "__main__":
    rng = np.random.default_rng(0)
    T = K = 1024
    x = rng.standard_normal((2, T, DIM), dtype=np.float32)
    ctx = rng.standard_normal((2, K, SRC), dtype=np.float32)
    Wq = rng.standard_normal((DIM, DIM), dtype=np.float32) / 32
    Wk = rng.standard_normal((DIM, SRC), dtype=np.float32) / 34
    Wv = rng.standard_normal((DIM, SRC), dtype=np.float32) / 34
    Wo = rng.standard_normal((DIM, DIM), dtype=np.float32) / 32
    z = np.zeros(DIM, dtype=np.float32)
    got = kernel(x, ctx, Wq, z, Wk, z, Wv, z, Wo, z)

    def ref(x, ctx):
        q = x @ Wq.T
        k = ctx @ Wk.T
        v = ctx @ Wv.T
        B = x.shape[0]
        q = q.reshape(B, T, NH, HD).transpose(0, 2, 1, 3)
        k = k.reshape(B, K, NH, HD).transpose(0, 2, 1, 3)
        v = v.reshape(B, K, NH, HD).transpose(0, 2, 1, 3)
        cos, sin = _rope_tables(T, HD)

        def rot_half(t):
            t1, t2 = t[..., ::2], t[..., 1::2]
            return np.stack((-t2, t1), axis=-1).reshape(t.shape)

        q = q * cos[None, None] + rot_half(q) * sin[None, None]
        k = k * cos[None, None] + rot_half(k) * sin[None, None]
        s = np.einsum("bhtd,bhkd->bhtk", q, k) / np.sqrt(HD)
        s = np.exp(s - s.max(-1, keepdims=True))
        w = s / s.sum(-1, keepdims=True)
        o = np.einsum("bhtk,bhkd->bhtd", w, v)
        o = o.transpose(0, 2, 1, 3).reshape(B, T, DIM)
        return o @ Wo.T

    want = ref(x, ctx)
    err = np.abs(got - want).max() / np.abs(want).max()
    print("smoke relerr:", err)


# revision 28
# speedup vs baseline: 1.0526x; 1.0526x over previous
"""CrossAttention Trainium2 kernel (8-core SPMD, batch x head-group sharded).

Problem (hardcoded): x (2,2048,1024) fp32, context (2,2048,1152) fp32,
Wq (1024,1024), Wk/Wv (1024,1152), Wo (1024,1024), zero biases.
16 heads x 64 dim, RoPE (interleaved rotate_half, cat-table), softmax over K,
out projection. Output (2, 2048, 1024) fp32.

Sharding: core c in 0..7 handles batch b = c//4 and head group g = c%4
(heads 4g..4g+3). Each core computes a partial y_c = attn(heads) @ Wo_slice;
host sums 4 partials per batch and adds bo.

v2 design (cost-model driven):
  - bf16 operands everywhere on the attention path (proj inputs/weights,
    q/k/v, softmax weights, attn out, Wo). Matmul cost = out-cols regardless
    of dtype; bf16 halves DMA + SBUF and permits 1024-wide moving operands.
  - RoPE via single projection + R*q rotation matmul on PE (R = 128x128
    block-diag rotate_half matrix) + 3 DVE combine ops per tile.
  - scores s[k128, t1024] = kT_h^T q_h per (head, kt);  exp on ACT (the
    133us critical path);  AV transposed: o[t128, 65] = wt_tile^T v_aug
    (65th v col = ones -> softmax denom Z), accumulated over kt.
  - normalize on DVE (1/Z per t-partition), PE-transpose o -> on[e, t],
    O-proj y[t, m] = on^T Wo, DVE copy, DMA out.
  - emission order software-pipelines everything: projections / V / O-proj
    / transposes are spliced into the exp-paced attention loops as PE
    fillers so PE and ACT both stay busy.
"""
import sys

sys.path.insert(0, "/opt/trn_rl_repo")

import numpy as np
import ml_dtypes
import concourse.bass as bass  # noqa: F401
import concourse.mybir as mybir
import concourse.tile as tile
from concourse import bacc
from concourse.bass_utils import run_bass_kernel_spmd

F32 = mybir.dt.float32
F32R = mybir.dt.float32r
BF16 = mybir.dt.bfloat16
AF = mybir.ActivationFunctionType
OP = mybir.AluOpType
BF = ml_dtypes.bfloat16

DIM = 1024
SRC = 1152
NH = 16
HD = 64
GH = 4          # heads per core
GD = GH * HD    # 256 projection dims per core
ROPE_BASE = 10000.0


# ---------------------------------------------------------------- host helpers
def _rope_tables(seq_len: int, head_dim: int):
    inv_freq = 1.0 / (ROPE_BASE ** (np.arange(0, head_dim, 2, dtype=np.float32) / head_dim))
    t = np.arange(seq_len, dtype=np.float32)
    freqs = t[:, None] * inv_freq[None, :]
    emb = np.concatenate([freqs, freqs], axis=-1)  # (L, 64)
    return np.cos(emb).astype(np.float32), np.sin(emb).astype(np.float32)


def _rot128():
    """128x128 rotate_half matrix for a 2-head partition chunk (interleaved)."""
    r64 = np.zeros((HD, HD), dtype=np.float32)
    for i in range(HD // 2):
        r64[2 * i, 2 * i + 1] = -1.0
        r64[2 * i + 1, 2 * i] = 1.0
    return np.kron(np.eye(2, dtype=np.float32), r64)   # (128, 128)


# ---------------------------------------------------------------- device build
def build_nc(T: int, K: int, n_cores: int = 8):
    assert T % 1024 == 0 and K % 512 == 0
    NTW = T // 1024         # 1024-wide t windows ("tw")
    NKT = K // 128          # 128-wide k tiles
    NKW = K // 512          # 512-wide k/proj windows
    NQW = T // 512          # 512-wide q proj windows
    NCC = DIM // 128        # x contraction chunks (8)
    NCS = SRC // 128        # context contraction chunks (9)

    nc = bacc.Bacc("TRN2", target_bir_lowering=False, debug=False,
                   num_devices=n_cores)

    xT = nc.declare_dram_parameter("xT", [DIM, T], BF16, isOutput=False)
    cT = nc.declare_dram_parameter("cT", [SRC, K], BF16, isOutput=False)
    wqT = nc.declare_dram_parameter("wqT", [DIM, GD], BF16, isOutput=False)
    wkT = nc.declare_dram_parameter("wkT", [SRC, GD], BF16, isOutput=False)
    wvT = nc.declare_dram_parameter("wvT", [SRC, GH * 65], BF16, isOutput=False)
    woT = nc.declare_dram_parameter("woT", [GD, DIM], BF16, isOutput=False)
    rT = nc.declare_dram_parameter("rT", [128, 128], F32R, isOutput=False)
    cosT = nc.declare_dram_parameter("cosT", [128, max(T, K)], BF16, isOutput=False)
    sinT = nc.declare_dram_parameter("sinT", [128, max(T, K)], BF16, isOutput=False)
    idn = nc.declare_dram_parameter("idn", [128, 128], BF16, isOutput=False)
    y = nc.declare_dram_parameter("y", [T, DIM], F32, isOutput=True)

    with tile.TileContext(nc) as tc:
        with (
            tc.tile_pool(name="consts", bufs=1) as consts,
            tc.tile_pool(name="resid", bufs=1) as resid,
            tc.tile_pool(name="rope", bufs=2) as rope,
            tc.tile_pool(name="wtp", bufs=20) as wtp,
            tc.tile_pool(name="osbp", bufs=1) as osbp,
            tc.tile_pool(name="zp", bufs=4) as zp,
            tc.tile_pool(name="yp", bufs=2) as yp,
            tc.tile_pool(name="pp", bufs=2, space="PSUM") as pp,
            tc.tile_pool(name="sp", bufs=2, space="PSUM") as sp,
            tc.tile_pool(name="avp", bufs=1, space="PSUM") as avp,
        ):
            # ---------------- constants (DMA order = arrival order) ---------
            # The serial DMA track gates the pipeline start: order transfers
            # by first use so the first exp fires as early as possible.
            rT_sb = consts.tile([128, 128], F32R, tag="rT")
            nc.sync.dma_start(out=rT_sb, in_=rT[:, :])
            wk_sb = consts.tile([128, NCS, GD], BF16, tag="wk")
            nc.sync.dma_start(out=wk_sb, in_=wkT[:, :].rearrange("(o p) f -> p o f", p=128))
            ct_sb = consts.tile([128, NCS, K], BF16, tag="ct")
            ct_dram = cT[:, :].rearrange("(o p) t -> p o t", p=128)

            def ct_win_dma(w):
                nc.sync.dma_start(out=ct_sb[:, :, w * 512:(w + 1) * 512],
                                  in_=ct_dram[:, :, w * 512:(w + 1) * 512])

            ct_win_dma(0)
            wv_sb = consts.tile([128, NCS, GH * 65], BF16, tag="wv")
            nc.sync.dma_start(out=wv_sb, in_=wvT[:, :].rearrange("(o p) f -> p o f", p=128))
            wq_sb = consts.tile([128, NCC, GD], BF16, tag="wq")
            nc.sync.dma_start(out=wq_sb, in_=wqT[:, :].rearrange("(o p) f -> p o f", p=128))
            cos_sb = consts.tile([128, max(T, K)], BF16, tag="cos")
            nc.sync.dma_start(out=cos_sb, in_=cosT[:, :])
            sin_sb = consts.tile([128, max(T, K)], BF16, tag="sin")
            nc.sync.dma_start(out=sin_sb, in_=sinT[:, :])
            x_sb = consts.tile([128, NCC, T], BF16, tag="x")
            x_dram = xT[:, :].rearrange("(o p) t -> p o t", p=128)

            def x_win_dma(w):
                nc.sync.dma_start(out=x_sb[:, :, w * 512:(w + 1) * 512],
                                  in_=x_dram[:, :, w * 512:(w + 1) * 512])

            x_win_dma(0)
            x_win_dma(1)
            ct_win_dma(1)
            ct_win_dma(2)
            ct_win_dma(3)
            wo_sb = consts.tile([128, 2, DIM], BF16, tag="wo")
            nc.sync.dma_start(out=wo_sb, in_=woT[:, :].rearrange("(o p) f -> p o f", p=128))
            idn_sb = consts.tile([128, 128], BF16, tag="idn")
            nc.sync.dma_start(out=idn_sb, in_=idn[:, :])
            for w in range(2, NQW):
                x_win_dma(w)

            # ---------------- residents ------------------------------------
            kT_sb = [resid.tile([128, K], BF16, tag=f"kT{m}", name=f"kT{m}") for m in range(2)]
            qT_sb = [resid.tile([128, T], BF16, tag=f"qT{m}", name=f"qT{m}") for m in range(2)]
            v_sb = [resid.tile([128, GH * 65], BF16, tag=f"v{kt}", name=f"v{kt}")
                    for kt in range(NKT)]
            on_sb = [resid.tile([128, T], BF16, tag=f"on{m}", name=f"on{m}") for m in range(2)]

            wt = {}     # (h, kt) -> wt tile [128, 1024] for current tw
            osb = {}    # tt -> [128, GD] tile for current tw

            # ---------------- emission helpers -----------------------------
            def rope_combine(dst, raw_sb, n):
                """dst[:, n*512:+512] (bf16) = raw*cos + (R raw)*sin."""
                t0 = n * 512
                rps = pp.tile([128, 512], F32, tag="pps", name="rps")
                nc.tensor.matmul(rps, rT_sb, raw_sb, start=True, stop=True)
                t1 = rope.tile([128, 512], F32, tag="t1")
                nc.vector.tensor_tensor(t1, raw_sb, cos_sb[:, t0:t0 + 512], OP.mult)
                t2 = rope.tile([128, 512], F32, tag="t2")
                nc.vector.tensor_tensor(t2, rps, sin_sb[:, t0:t0 + 512], OP.mult)
                nc.vector.tensor_tensor(dst[:, t0:t0 + 512], t1, t2, OP.add)

            def proj_k(m, n):
                kps = pp.tile([128, 512], F32, tag="pps", name="kps")
                for c in range(NCS):
                    nc.tensor.matmul(kps, wk_sb[:, c, m * 128:(m + 1) * 128],
                                     ct_sb[:, c, n * 512:(n + 1) * 512],
                                     start=(c == 0), stop=(c == NCS - 1))
                ksb = rope.tile([128, 512], F32R, tag="ksb")
                nc.vector.tensor_copy(ksb, kps)
                rope_combine(kT_sb[m], ksb, n)

            def proj_q(m, n):
                qps = pp.tile([128, 512], F32, tag="pps", name="qps")
                for c in range(NCC):
                    nc.tensor.matmul(qps, wq_sb[:, c, m * 128:(m + 1) * 128],
                                     x_sb[:, c, n * 512:(n + 1) * 512],
                                     start=(c == 0), stop=(c == NCC - 1))
                qsb = rope.tile([128, 512], F32R, tag="ksb", name="qsb")
                nc.vector.tensor_copy(qsb, qps)
                rope_combine(qT_sb[m], qsb, n)

            def proj_v(kt):
                vps = pp.tile([128, GH * 65], F32, tag="pps", name="vps")
                for c in range(NCS):
                    nc.tensor.matmul(vps, ct_sb[:, c, kt * 128:(kt + 1) * 128],
                                     wv_sb[:, c, :], start=(c == 0), stop=(c == NCS - 1))
                nc.vector.tensor_copy(v_sb[kt], vps)
                ones_ap = v_sb[kt].rearrange("p (h e) -> p h e", h=GH)[:, :, 64]
                nc.vector.memset(ones_ap, 1.0)

            def emit_scores(tw, h, kt):
                """One scores matmul pair + exp for (head h, t-window tw, k-tile kt)."""
                m, off = h // 2, (h % 2) * 64
                sps = sp.tile([128, 1024], F32, tag="sps", name="sps")
                for half in range(2):
                    nc.tensor.matmul(
                        sps[:, half * 512:(half + 1) * 512],
                        kT_sb[m][off:off + 64, kt * 128:(kt + 1) * 128],
                        qT_sb[m][off:off + 64, tw * 1024 + half * 512:tw * 1024 + (half + 1) * 512],
                        start=True, stop=True)
                wt_t = wtp.tile([128, 1024], BF16, tag="wt", name="wt")
                nc.scalar.activation(wt_t, sps, AF.Exp, scale=0.125)
                wt[(h, kt)] = wt_t

            def emit_av_kt(h, kt, avA, avB):
                """Accumulate o[t, 65] for all 8 tts of one kt (JIT per-kt).

                Tiles are pre-zeroed by DVE memset; matmuls use start=False so
                per-element has_written bits drive accumulate-vs-overwrite.
                """
                for tt in range(8):
                    at = avA if tt < 4 else avB
                    col = (tt % 4) * 65
                    nc.tensor.matmul(
                        at[:, col:col + 65],
                        wt[(h, kt)][:, tt * 128:(tt + 1) * 128],
                        v_sb[kt][:, h * 65:(h + 1) * 65],
                        start=False, stop=(kt == NKT - 1),
                        skip_group_check=True)

            def emit_norm(tw, h, av_tile, tts):
                """osb[tt][:, h-cols] = o / Z for the 4 tts of one AV pass."""
                zsb = zp.tile([128, 4], F32, tag="zsb", name="zsb")
                for j, tt in enumerate(tts):
                    col = (tt % 4) * 65
                    nc.vector.tensor_copy(zsb[:, j:j + 1], av_tile[:, col + 64:col + 65])
                rec = zp.tile([128, 4], F32, tag="rec", name="rec")
                nc.vector.reciprocal(rec, zsb)
                for j, tt in enumerate(tts):
                    if (tw, tt) not in osb:
                        ot = osbp.tile([128, GD], BF16, tag=f"osb{tw}_{tt}",
                                       name=f"osb{tw}_{tt}")
                        osb[(tw, tt)] = ot
                    col = (tt % 4) * 65
                    nc.vector.tensor_scalar(
                        osb[(tw, tt)][:, h * 64:(h + 1) * 64],
                        av_tile[:, col:col + 64],
                        rec[:, j:j + 1], None, OP.mult)

            def emit_transp(tw, m, tt, ot):
                # transpose psum borrows the short-lived proj/yps bank pair
                tps = pp.tile([128, 128], BF16, tag="pps", name="tps")
                nc.tensor.transpose(tps, ot[:, m * 128:(m + 1) * 128], idn_sb)
                nc.vector.tensor_copy(
                    on_sb[m][:, tw * 1024 + tt * 128:tw * 1024 + (tt + 1) * 128], tps)

            def emit_oproj(tw, tt, use_act=False):
                gtt = tw * 8 + tt
                for nn in range(2):
                    yps = pp.tile([128, 512], F32, tag="pps", name="yps")
                    for dc in range(2):
                        nc.tensor.matmul(yps, on_sb[dc][:, gtt * 128:(gtt + 1) * 128],
                                         wo_sb[:, dc, nn * 512:(nn + 1) * 512],
                                         start=(dc == 0), stop=(dc == 1))
                    yt = yp.tile([128, 512], F32, tag="yt", name="yt")
                    if use_act:
                        nc.scalar.copy(yt, yps)   # ACT is idle post-exp (tail)
                    else:
                        nc.vector.tensor_copy(yt, yps)
                    nc.sync.dma_start(out=y[gtt * 128:(gtt + 1) * 128, nn * 512:(nn + 1) * 512],
                                      in_=yt)

            # ---------------- filler queue ----------------------------------
            # Deferred PE work, consumed in FIFO order with a cycle budget.
            # ensure_*() force-drains through a key so emission order always
            # places producers before consumers (tile deps follow emission
            # order, so this is a correctness requirement, not just pacing).
            fillers = []          # list of (key, cost_cycles, fn)
            done_keys = set()
            debt = [0.0]

            def pop_filler(charge=True):
                key, cost, fn = fillers.pop(0)
                fn()
                done_keys.add(key)
                if charge:
                    debt[0] -= cost

            def emit_fillers(budget):
                # cap so long filler droughts don't bank unbounded credit
                debt[0] = min(debt[0] + budget, 6000.0)
                while fillers and fillers[0][1] <= debt[0]:
                    pop_filler()

            def ensure(keys):
                # forced pops are real PE time already spent out-of-budget;
                # don't let them starve later budget-based draining
                missing = [k for k in keys if k not in done_keys]
                while missing:
                    if not fillers:
                        raise AssertionError(f"unsatisfiable deps: {missing}")
                    pop_filler(charge=False)
                    missing = [k for k in missing if k not in done_keys]

            def q_keys(m, tw):
                return [("Q", m, n) for n in range(2 * tw, 2 * tw + 2)]

            def v_keys():
                return [("V", kt) for kt in range(NKT)]

            def scores_guarded(tw, h, kt):
                m = h // 2
                ensure([("K", m, kt // 4)])
                ensure(q_keys(m, tw))
                emit_scores(tw, h, kt)

            # ---------------- schedule --------------------------------------
            # Prefix: first scores' deps + work done in the DMA shadow.
            proj_k(0, 0)
            done_keys.add(("K", 0, 0))
            for kt in range(4):
                proj_v(kt)
                done_keys.add(("V", kt))
            proj_k(1, 0)
            done_keys.add(("K", 1, 0))
            for n in range(2):
                proj_q(0, n)
                done_keys.add(("Q", 0, n))

            # Static fillers, interleaved to match DMA arrival order.
            for kt in range(4, NKT):
                fillers.append((("V", kt), 9 * 260, lambda kt=kt: proj_v(kt)))
                if kt in (5, 8, 11):   # K m0 windows 1..3 land between Vs
                    n = {5: 1, 8: 2, 11: 3}[kt]
                    fillers.append((("K", 0, n), 10 * 512, lambda n=n: proj_k(0, n)))
            for n in range(1, NKW):
                fillers.append((("K", 1, n), 10 * 512, lambda n=n: proj_k(1, n)))
            for n in range(2):
                fillers.append((("Q", 1, n), 9 * 512, lambda n=n: proj_q(1, n)))
            for twl in range(1, NTW):
                for m in range(2):
                    for n in range(twl * 2, twl * 2 + 2):
                        fillers.append((("Q", m, n), 9 * 512,
                                        lambda m=m, n=n: proj_q(m, n)))

            heads = [(tw, h) for tw in range(NTW) for h in range(GH)]

            def head_tail(tw, h, avA, avB):
                """Norm + transpose/oproj bookkeeping after a head's AV."""
                emit_norm(tw, h, avA, [0, 1, 2, 3])
                emit_norm(tw, h, avB, [4, 5, 6, 7])
                last = tw == NTW - 1
                if h == 1:
                    for tt in range(8):
                        fillers.append((("T", tw, 0, tt), 128,
                                        lambda tw=tw, tt=tt, ot=osb[(tw, tt)]:
                                        emit_transp(tw, 0, tt, ot)))
                elif h == 3:
                    for tt in range(8):
                        fillers.append((("T", tw, 1, tt), 128,
                                        lambda tw=tw, tt=tt, ot=osb[(tw, tt)]:
                                        emit_transp(tw, 1, tt, ot)))
                    for tt in range(8):
                        fillers.append((("O", tw, tt), 4 * 512 + 600,
                                        lambda tw=tw, tt=tt, last=last:
                                        emit_oproj(tw, tt, use_act=last)))

            # AV for head i runs lagged, inside block i+1 (wt tiles banked).
            for i, (tw, h) in enumerate(heads):
                prev = heads[i - 1] if i > 0 else None
                if prev is not None:
                    avA = avp.tile([128, 4 * 65], F32, tag="avA", name="avA")
                    avB = avp.tile([128, 4 * 65], F32, tag="avB", name="avB")
                    nc.vector.memset(avA, 0.0)
                    nc.vector.memset(avB, 0.0)
                for kt in range(NKT):
                    scores_guarded(tw, h, kt)
                    if prev is not None:
                        ensure([("V", kt)])
                        emit_av_kt(prev[1], kt, avA, avB)
                    if kt == 10 and i + 1 < len(heads):
                        ensure(q_keys(heads[i + 1][1] // 2, heads[i + 1][0]))
                    emit_fillers(1400 if i == 0 else 950)
                if prev is not None:
                    head_tail(prev[0], prev[1], avA, avB)

            # tail: AV + norm for the final head, then remaining fillers
            tw, h = heads[-1]
            avA = avp.tile([128, 4 * 65], F32, tag="avA", name="avA")
            avB = avp.tile([128, 4 * 65], F32, tag="avB", name="avB")
            nc.vector.memset(avA, 0.0)
            nc.vector.memset(avB, 0.0)
            for kt in range(NKT):
                emit_av_kt(h, kt, avA, avB)
            head_tail(tw, h, avA, avB)

            while fillers:
                pop_filler()

    nc.compile()
    return nc


# ---------------------------------------------------------------- host wrapper
def make_in_maps(x, context, Wq, Wk, Wv, Wo, n_cores=8):
    B, T, _ = x.shape
    K = context.shape[1]
    cos, sin = _rope_tables(max(T, K), HD)      # (L, 64)
    cosT = np.ascontiguousarray(np.tile(cos.T, (2, 1)))   # (128, L)
    sinT = np.ascontiguousarray(np.tile(sin.T, (2, 1)))
    rt = np.ascontiguousarray(_rot128().T)

    in_maps = []
    for c in range(n_cores):
        b, g = c // 4, c % 4
        sl = slice(g * GD, (g + 1) * GD)
        wvTa = np.zeros((SRC, GH * 65), dtype=np.float32)
        for h in range(GH):
            wvTa[:, h * 65:h * 65 + 64] = Wv[g * GD + h * HD: g * GD + (h + 1) * HD, :].T
        in_maps.append({
            "xT": np.ascontiguousarray(x[b].T).astype(BF),
            "cT": np.ascontiguousarray(context[b].T).astype(BF),
            "wqT": np.ascontiguousarray(Wq[sl, :].T).astype(BF),
            "wkT": np.ascontiguousarray(Wk[sl, :].T).astype(BF),
            "wvT": wvTa.astype(BF),
            "woT": np.ascontiguousarray(Wo[:, sl].T).astype(BF),
            "rT": rt,
            "cosT": cosT.astype(BF), "sinT": sinT.astype(BF),
            "idn": np.eye(128, dtype=np.float32).astype(BF),
        })
    return in_maps


def run(nc, in_maps, n_cores=8):
    res = run_bass_kernel_spmd(nc, in_maps, core_ids=list(range(n_cores)))
    return res.results


def kernel(x, context, Wq, bq, Wk, bk, Wv, bv, Wo, bo):
    B, T, _ = x.shape
    K = context.shape[1]
    x = np.asarray(x, dtype=np.float32)
    context = np.asarray(context, dtype=np.float32)
    Wq, Wk, Wv, Wo = (np.asarray(a, dtype=np.float32) for a in (Wq, Wk, Wv, Wo))
    bq, bk, bv, bo = (np.asarray(a, dtype=np.float32) for a in (bq, bk, bv, bo))

    nc = build_nc(T, K, n_cores=8)
    in_maps = make_in_maps(x, context, Wq, Wk, Wv, Wo)
    assert not bq.any() and not bk.any() and not bv.any(), "nonzero qkv bias unsupported"
    results = run(nc, in_maps)

    out = np.zeros((B, T, DIM), dtype=np.float32)
    for c in range(8):
        out[c // 4] += results[c]["y"]
    out += bo[None, None, :]
    return out


if __name__ == "__main__":
    rng = np.random.default_rng(0)
    T = K = 1024
    x = rng.standard_normal((2, T, DIM), dtype=np.float32)
    ctx = rng.standard_normal((2, K, SRC), dtype=np.float32)
    Wq = rng.standard_normal((DIM, DIM), dtype=np.float32) / 32
    Wk = rng.standard_normal((DIM, SRC), dtype=np.float32) / 34
    Wv = rng.standard_normal((DIM, SRC), dtype=np.float32) / 34
    Wo = rng.standard_normal((DIM, DIM), dtype=np.float32) / 32
    z = np.zeros(DIM, dtype=np.float32)
    got = kernel(x, ctx, Wq, z, Wk, z, Wv, z, Wo, z)

    def ref(x, ctx):
        q = x @ Wq.T
        k = ctx @ Wk.T
        v = ctx @ Wv.T
        B = x.shape[0]
        q = q.reshape(B, T, NH, HD).transpose(0, 2, 1, 3)
        k = k.reshape(B, K, NH, HD).transpose(0, 2, 1, 3)
        v = v.reshape(B, K, NH, HD).transpose(0, 2, 1, 3)
        cos, sin = _rope_tables(T, HD)

        def rot_half(t):
            t1, t2 = t[..., ::2], t[..., 1::2]
            return np.stack((-t2, t1), axis=-1).reshape(t.shape)

        q = q * cos[None, None] + rot_half(q) * sin[None, None]
        k = k * cos[None, None] + rot_half(k) * sin[None, None]
        s = np.einsum("bhtd,bhkd->bhtk", q, k) / np.sqrt(HD)
        s = np.exp(s - s.max(-1, keepdims=True))
        w = s / s.sum(-1, keepdims=True)
        o = np.einsum("bhtk,bhkd->bhtd", w, v)
        o = o.transpose(0, 2, 1, 3).reshape(B, T, DIM)
        return o @ Wo.T

    want = ref(x, ctx)
    err = np.abs(got - want).max() / np.abs(want).max()
    print("smoke relerr:", err)
